# revision 45
# baseline (speedup 1.0000x reference)
"""Trainium2 Bass kernel for the AMTCL loss (nn_AMTCL_66520453480770).

Math: the reference builds a [B,B] pairwise distance matrix dist[i,j] between
inputs[i] and centers[targets[j]] (weights 2**centers_weights[targets[j]]).
Since dist[i,j] depends on j only through c = targets[j], the whole problem
collapses to the [B,C] matrix
    dc2[i,c] = sum_d w2[c,d] * (centers[c,d] - inputs[i,d])**2
with
    dist_ap[i] = sqrt(dc2[i, t_i])            (all same-class j are equal)
    dist_an[i] = sqrt(min_{c present, c != t_i} dc2[i,c])   (sqrt commutes
                 with min, so mining happens on squared distances)
    cc[i]      = centers_dist[t_i],  centers_dist[c] = sqrt(min_{j!=c} cd2[c,j])
    loss       = mean(dist_ap + relu(cc - dist_an))
This is exactly equal to the reference (40x less compute than the B^2 form);
GEMMs run in bf16 with fp32 PSUM accumulation (loss rel err ~1e-4).

dc2 is one GEMM with contraction K = 2D+1:
    dc2[i,c] = sum_d xsq[i,d]*w2[c,d] + sum_d x[i,d]*(-2*w2[c,d]*c[c,d]) + a[c]
The a[c] row rides in as a K=1 term; the cd2 GEMM shares the same center-side
operands and adds 2^40*I via an identity matmul to mask the diagonal.
Mining reads dc2 straight out of PSUM per 128-anchor chunk, overlapped with
the next chunk's matmuls.

Sharding: data-parallel over the 4096 anchor rows -> 8 cores x 512 rows.
centers/centers_weights replicated. Each core emits a partial loss sum [1,1];
the host sums the 8 scalars and divides by B.
"""

import ml_dtypes
import numpy as np

import concourse.bass as bass
import concourse.bacc as bacc
import concourse.mybir as mybir
import concourse.tile as tile
from concourse.bass_utils import run_bass_kernel_spmd

B, C, D = 4096, 100, 384
NCORES = 8
ROWS = B // NCORES          # 512 anchor rows per core
MCH = ROWS // 128           # 4 partition chunks of anchor rows
KD = D // 128               # 3 partition chunks of the feature dim
PEN = float(2 ** 40)        # self-class / absent-class / diagonal penalty
LN2 = float(np.log(2.0))
F32 = mybir.dt.float32
BF16 = mybir.dt.bfloat16
AF = mybir.ActivationFunctionType
ALU = mybir.AluOpType

# bf16 input block layout (columns); iota/eye/ones built on device
T_O = 0                      # targets column-chunks [128, MCH]
CWT_O = T_O + MCH            # centers_weights.T chunks (gates the Exp)
CT_O = CWT_O + KD * C        # centers.T chunks [128, 3*100]
XT_O = CT_O + KD * C         # x.T shard, anchor-chunk-major
BW = XT_O + KD * ROWS        # 2140

# f32 input row: absent-class penalty row (normally all zeros)
FW = C


def build_nc() -> bass.Bass:
    nc = bacc.Bacc(
        "TRN2", target_bir_lowering=False, debug=False, num_devices=NCORES
    )

    bin_ = nc.declare_dram_parameter("bin", [128, BW], BF16, isOutput=False)
    fin = nc.declare_dram_parameter("fin", [1, FW], F32, isOutput=False)
    out = nc.declare_dram_parameter("out", [1, 1], F32, isOutput=True)

    with tile.TileContext(nc) as tc:
        with (
            tc.tile_pool(name="wts", bufs=1) as wp,
            tc.tile_pool(name="work", bufs=2) as kp,
            tc.tile_pool(name="ps1", bufs=1, space="PSUM") as pp1,
            tc.tile_pool(name="ps2", bufs=3, space="PSUM") as pp2,
        ):
            # consts+centers land first (unblock prologue); x in 3 chunks.
            # Each dma_start's descriptor generation serializes on its
            # issuing sequencer (~2-3us for a [128,*] transfer), so spread
            # the loads across idle engines' DGE paths to issue in parallel.
            bsb = wp.tile([128, XT_O], BF16, tag="bsb")
            nc.sync.dma_start(bsb[:], bin_[:, 0:XT_O])
            fsb = wp.tile([1, FW], F32, tag="fsb")
            nc.sync.dma_start(fsb[:], fin[:])
            # x arrives per ANCHOR chunk (m-major): chunk m's GEMM only
            # waits for its own 98KB, not the whole shard
            xtiles = []
            for m in range(MCH):
                xm = wp.tile([128, KD * 128], BF16, tag=f"xm{m}")
                # last chunk issues from the scalar HWDGE so its descriptor
                # generation overlaps the sync sequencer's
                eng = nc.scalar if m == MCH - 1 else nc.sync
                eng.dma_start(
                    xm[:], bin_[:, XT_O + m * KD * 128 :
                                 XT_O + (m + 1) * KD * 128]
                )
                xtiles.append(xm)

            t_b = bsb[:, T_O : T_O + MCH]
            ct_b = bsb[:, CT_O : CT_O + KD * C]
            cwt_b = bsb[:, CWT_O : CWT_O + KD * C]
            penrow_f = fsb[0:1, 0:C]

            # ---- constants built on device (gpsimd is idle early) ----
            ones_b = wp.tile([128, 128], BF16, tag="ones_b")
            nc.gpsimd.memset(ones_b[:], 1.0)
            ones_f = wp.tile([128, 1], F32, tag="ones_f")
            nc.gpsimd.memset(ones_f[:], 1.0)
            iota_i = wp.tile([128, C], mybir.dt.int32, tag="iota_i")
            nc.gpsimd.iota(iota_i[:], pattern=[[1, C]], base=0,
                           channel_multiplier=0)
            iota_b = wp.tile([128, C], BF16, tag="iota_b")
            nc.gpsimd.tensor_copy(iota_b[:], iota_i[:])
            eye_b = wp.tile([C, C], BF16, tag="eye_b")
            nc.gpsimd.affine_select(
                eye_b[:], ones_b[0:C, 0:C], pattern=[[1, C]],
                compare_op=ALU.is_equal, fill=0.0, base=0,
                channel_multiplier=-1,
            )
            penfull = wp.tile([C, C], BF16, tag="penfull")
            nc.gpsimd.memset(penfull[:], PEN)
            eyepen_b = wp.tile([C, C], BF16, tag="eyepen_b")
            nc.gpsimd.affine_select(
                eyepen_b[:], penfull[:], pattern=[[1, C]],
                compare_op=ALU.is_equal, fill=0.0, base=0,
                channel_multiplier=-1,
            )

            # PE warm-up: HAM clock-gates a cold PE to 1.2GHz and needs
            # ~3.4us of sustained activity to ungate. Burn dummy matmuls on
            # the all-ones tile during the otherwise-dead DMA wait so the
            # real GEMMs run at 2.4GHz.
            warm_ps = pp1.tile([128, C], F32, tag="loss")
            for _ in range(24):
                nc.tensor.matmul(warm_ps[:], ones_b[:], ones_b[:, 0:C])

            # ---- center-side prep (bf16) ----
            w2b = wp.tile([128, KD * C], BF16, tag="w2b")
            nc.scalar.activation(w2b[:], cwt_b, AF.Exp, scale=LN2)
            # dummy sqrt: pulls the sqrt ACT table load off the critical
            # path; reads w2b so it lands after the Exp (exp-table) use.
            sqdummy = wp.tile([1, 1], F32, tag="sqdummy")
            nc.scalar.sqrt(sqdummy[:], w2b[0:1, 0:1])

            csqb = wp.tile([128, KD * C], BF16, tag="csqb")
            nc.scalar.square(csqb[:], ct_b)
            cm2b = wp.tile([128, KD * C], BF16, tag="cm2b")
            nc.vector.tensor_scalar(cm2b[:], ct_b, -2.0, None, op0=ALU.mult)
            m2b = wp.tile([128, KD * C], BF16, tag="m2b")
            nc.vector.tensor_tensor(m2b[:], w2b[:], cm2b[:], op=ALU.mult)
            wsqb = wp.tile([128, KD * C], BF16, tag="wsqb")
            nc.vector.tensor_tensor(wsqb[:], w2b[:], csqb[:], op=ALU.mult)

            # one-hot masks (only need iota/t -> very early)
            ohw = wp.tile([128, MCH * C], F32, tag="ohw")
            oh3 = ohw[:].rearrange("p (m c) -> p m c", c=C)
            nc.vector.tensor_tensor(
                oh3, iota_b[:, None, :].broadcast_to([128, MCH, C]),
                t_b[:, :, None].broadcast_to([128, MCH, C]), op=ALU.is_equal
            )
            ohpw = wp.tile([128, MCH * C], F32, tag="ohpw")
            nc.vector.tensor_scalar(ohpw[:], ohw[:], PEN, None, op0=ALU.mult)

            # a[c] = sum_d w2*c^2, + absent-class penalty row -> bf16
            psum_arow = pp1.tile([1, C], F32, tag="arow")
            for k in range(KD):
                nc.tensor.matmul(
                    psum_arow[:], ones_b[:, 0:1],
                    wsqb[:, k * C : (k + 1) * C],
                    start=(k == 0), stop=(k == KD - 1),
                )
            arowb = wp.tile([1, C], BF16, tag="arowb")
            nc.vector.tensor_tensor(
                arowb[:], psum_arow[:], penrow_f, op=ALU.add
            )

            # ---- cd2 GEMM [100,100]: shared center operands + PEN*I ----
            psum_cd2 = pp1.tile([C, C], F32, tag="cd2")
            for k in range(KD):
                nc.tensor.matmul(
                    psum_cd2[:], m2b[:, k * C : (k + 1) * C],
                    ct_b[:, k * C : (k + 1) * C],
                    start=(k == 0), stop=False,
                )
                nc.tensor.matmul(
                    psum_cd2[:], w2b[:, k * C : (k + 1) * C],
                    csqb[:, k * C : (k + 1) * C],
                    start=False, stop=False,
                )
            nc.tensor.matmul(
                psum_cd2[:], eyepen_b[:], eye_b[:],
                start=False, stop=False,
            )
            nc.tensor.matmul(
                psum_cd2[:], arowb[:], ones_b[0:1, 0:C],
                start=False, stop=True,
            )
            # min over j, then clip at 0 (max is monotone, so this equals
            # the reference's clip-then-min) -- tiny [C,1] clip
            cdmin2 = wp.tile([C, 1], F32, tag="cdmin2")
            nc.vector.tensor_reduce(
                cdmin2[:], psum_cd2[:], axis=mybir.AxisListType.X, op=ALU.min
            )
            cdmin2c = wp.tile([C, 1], F32, tag="cdmin2c")
            nc.vector.tensor_scalar(cdmin2c[:], cdmin2[:], 0.0, None,
                                    op0=ALU.max)
            cdminb = wp.tile([C, 1], BF16, tag="cdminb")
            nc.scalar.sqrt(cdminb[:], cdmin2c[:])
            # transpose to a row, broadcast down 128 partitions
            psum_cdrow = pp1.tile([1, C], F32, tag="cdrow")
            nc.tensor.matmul(psum_cdrow[:], cdminb[:], eye_b[:])
            cdrowb = wp.tile([1, C], BF16, tag="cdrowb")
            nc.vector.tensor_copy(cdrowb[:], psum_cdrow[:])
            psum_bc = pp1.tile([128, C], F32, tag="bcast")
            nc.tensor.matmul(psum_bc[:], ones_b[0:1, :], cdrowb[:])
            cdbf = wp.tile([128, C], F32, tag="cdbf")
            nc.vector.tensor_copy(cdbf[:], psum_bc[:])

            # cc[i] = centers_dist[t_i] (dc2-independent -> overlapped)
            cctw = kp.tile([128, MCH * C], F32, tag="cctw")
            nc.vector.tensor_tensor(
                cctw[:].rearrange("p (m c) -> p m c", c=C),
                cdbf[:, None, :].broadcast_to([128, MCH, C]), oh3,
                op=ALU.mult,
            )
            cc4 = kp.tile([128, MCH], F32, tag="cc4")
            nc.vector.tensor_reduce(
                cc4[:], cctw[:].rearrange("p (m c) -> p m c", c=C),
                axis=mybir.AxisListType.X, op=ALU.add,
            )

            # x^2 per anchor chunk (pipelines with the x DMA chunks)
            xsqtiles = []
            for m in range(MCH):
                xsq = wp.tile([128, KD * 128], BF16, tag=f"xsq{m}")
                nc.scalar.square(xsq[:], xtiles[m][:])
                xsqtiles.append(xsq)

            # ---- big GEMM + per-chunk mining straight out of PSUM ----
            an2 = kp.tile([128, MCH], F32, tag="an2")
            ap2 = kp.tile([128, MCH], F32, tag="ap2")
            for m in range(MCH):
                psum_dc2 = pp2.tile([128, C], F32, tag="dc2")
                for k in range(KD):
                    nc.tensor.matmul(
                        psum_dc2[:],
                        xtiles[m][:, k * 128 : (k + 1) * 128],
                        m2b[:, k * C : (k + 1) * C],
                        start=(k == 0), stop=False,
                    )
                for k in range(KD):
                    nc.tensor.matmul(
                        psum_dc2[:],
                        xsqtiles[m][:, k * 128 : (k + 1) * 128],
                        w2b[:, k * C : (k + 1) * C],
                        start=False, stop=False,
                    )
                nc.tensor.matmul(
                    psum_dc2[:], ones_b[0:1, :], arowb[:],
                    start=False, stop=True,
                )
                antm = kp.tile([128, C], F32, tag="antm")
                nc.vector.tensor_tensor(
                    antm[:], psum_dc2[:], ohpw[:, m * C : (m + 1) * C],
                    op=ALU.add,
                )
                nc.vector.tensor_reduce(
                    an2[:, m : m + 1], antm[:],
                    axis=mybir.AxisListType.X, op=ALU.min,
                )
                aptm = kp.tile([128, C], F32, tag="aptm")
                nc.vector.tensor_tensor(
                    aptm[:], psum_dc2[:], ohw[:, m * C : (m + 1) * C],
                    op=ALU.mult,
                )
                nc.vector.tensor_reduce(
                    ap2[:, m : m + 1], aptm[:],
                    axis=mybir.AxisListType.X, op=ALU.add,
                )

            # ---- loss_i = sqrt(ap2) + relu(cc - sqrt(an2)) ----
            an = kp.tile([128, MCH], F32, tag="an")
            nc.scalar.sqrt(an[:], an2[:])
            mrgin = kp.tile([128, MCH], F32, tag="mrgin")
            nc.vector.tensor_tensor(mrgin[:], cc4[:], an[:], op=ALU.subtract)
            mrg = kp.tile([128, MCH], F32, tag="mrg")
            relusum = kp.tile([128, 1], F32, tag="relusum")
            nc.scalar.activation(mrg[:], mrgin[:], AF.Relu,
                                 accum_out=relusum[:])
            ap = kp.tile([128, MCH], F32, tag="ap")
            apsum = kp.tile([128, 1], F32, tag="apsum")
            nc.scalar.activation(ap[:], ap2[:], AF.Sqrt, accum_out=apsum[:])
            losscol = kp.tile([128, 1], F32, tag="losscol")
            nc.vector.tensor_tensor(
                losscol[:], relusum[:], apsum[:], op=ALU.add
            )

            psum_loss = pp1.tile([1, 1], F32, tag="loss")
            nc.tensor.matmul(psum_loss[:], ones_f[:], losscol[:])
            res_sb = wp.tile([1, 1], F32, tag="res")
            nc.vector.tensor_copy(res_sb[:], psum_loss[:])
            nc.sync.dma_start(out[:], res_sb[:])

    nc.compile()
    return nc


_NC_CACHE: list = []


def _get_nc() -> bass.Bass:
    if not _NC_CACHE:
        _NC_CACHE.append(build_nc())
    return _NC_CACHE[0]


def make_in_maps(inputs, centers, centers_weights, targets):
    x = np.asarray(inputs, dtype=np.float32)
    c = np.asarray(centers, dtype=np.float32)
    cw = np.asarray(centers_weights, dtype=np.float32)
    t = np.asarray(targets).astype(np.int64)

    bconst = np.zeros((128, BW), dtype=np.float32)
    cT = c.T.reshape(KD, 128, C)
    cwT = cw.T.reshape(KD, 128, C)
    for k in range(KD):
        bconst[:, CT_O + k * C : CT_O + (k + 1) * C] = cT[k]
        bconst[:, CWT_O + k * C : CWT_O + (k + 1) * C] = cwT[k]

    fshared = np.zeros((1, FW), dtype=np.float32)
    present = np.zeros(C, dtype=bool)
    present[np.unique(t)] = True
    fshared[0, 0:C] = np.where(present, 0.0, PEN)

    xT = np.ascontiguousarray(x.T)                      # [D, B]

    in_maps = []
    for i in range(NCORES):
        rows = slice(i * ROWS, (i + 1) * ROWS)
        bcst = bconst.copy()
        # [m, p, k*128+a]: anchor-chunk-major packing of x.T
        xr = xT[:, rows].reshape(KD, 128, MCH, 128).transpose(2, 1, 0, 3)
        for m in range(MCH):
            bcst[:, XT_O + m * KD * 128 : XT_O + (m + 1) * KD * 128] = (
                xr[m].reshape(128, KD * 128)
            )
        ts = t[rows].astype(np.float32).reshape(MCH, 128)
        bcst[:, T_O : T_O + MCH] = ts.T
        in_maps.append({
            "bin": bcst.astype(ml_dtypes.bfloat16),
            "fin": fshared,
        })
    return in_maps


def kernel(inputs, centers, centers_weights, targets, epoch_number=None,
           **_ignored):
    nc = _get_nc()
    in_maps = make_in_maps(inputs, centers, centers_weights, targets)
    res = run_bass_kernel_spmd(nc, in_maps, core_ids=list(range(NCORES)))
    total = sum(float(r["out"][0, 0]) for r in res.results)
    return np.float32(total / B)


# revision 50
# speedup vs baseline: 1.0002x; 1.0002x over previous
"""Trainium2 Bass kernel for the AMTCL loss (nn_AMTCL_66520453480770).

Math: the reference builds a [B,B] pairwise distance matrix dist[i,j] between
inputs[i] and centers[targets[j]] (weights 2**centers_weights[targets[j]]).
Since dist[i,j] depends on j only through c = targets[j], the whole problem
collapses to the [B,C] matrix
    dc2[i,c] = sum_d w2[c,d] * (centers[c,d] - inputs[i,d])**2
with
    dist_ap[i] = sqrt(dc2[i, t_i])            (all same-class j are equal)
    dist_an[i] = sqrt(min_{c present, c != t_i} dc2[i,c])   (sqrt commutes
                 with min, so mining happens on squared distances)
    cc[i]      = centers_dist[t_i],  centers_dist[c] = sqrt(min_{j!=c} cd2[c,j])
    loss       = mean(dist_ap + relu(cc - dist_an))
This is exactly equal to the reference (40x less compute than the B^2 form);
GEMMs run in bf16 with fp32 PSUM accumulation (loss rel err ~1e-4).

dc2 is one GEMM with contraction K = 2D+1:
    dc2[i,c] = sum_d xsq[i,d]*w2[c,d] + sum_d x[i,d]*(-2*w2[c,d]*c[c,d]) + a[c]
The a[c] row rides in as a K=1 term; the cd2 GEMM shares the same center-side
operands and adds 2^40*I via an identity matmul to mask the diagonal.
Mining reads dc2 straight out of PSUM per 128-anchor chunk, overlapped with
the next chunk's matmuls.

Sharding: data-parallel over the 4096 anchor rows -> 8 cores x 512 rows.
centers/centers_weights replicated. Each core emits a partial loss sum [1,1];
the host sums the 8 scalars and divides by B.
"""

import ml_dtypes
import numpy as np

import concourse.bass as bass
import concourse.bacc as bacc
import concourse.mybir as mybir
import concourse.tile as tile
from concourse.bass_utils import run_bass_kernel_spmd

B, C, D = 4096, 100, 384
NCORES = 8
ROWS = B // NCORES          # 512 anchor rows per core
MCH = ROWS // 128           # 4 partition chunks of anchor rows
KD = D // 128               # 3 partition chunks of the feature dim
PEN = float(2 ** 40)        # self-class / absent-class / diagonal penalty
LN2 = float(np.log(2.0))
F32 = mybir.dt.float32
BF16 = mybir.dt.bfloat16
AF = mybir.ActivationFunctionType
ALU = mybir.AluOpType

# bf16 input block layout (columns); iota/eye/ones built on device.
# cwt first: it alone gates the Exp -> smaller first DMA lands sooner.
CWT_O = 0                    # centers_weights.T chunks (gates the Exp)
CT_O = CWT_O + KD * C        # centers.T chunks [128, 3*100]
T_O = CT_O + KD * C          # targets column-chunks [128, MCH]
XT_O = T_O + MCH             # x.T shard, anchor-chunk-major
BW = XT_O + KD * ROWS        # 2140

# f32 input row: absent-class penalty row (normally all zeros)
FW = C


def build_nc() -> bass.Bass:
    nc = bacc.Bacc(
        "TRN2", target_bir_lowering=False, debug=False, num_devices=NCORES
    )

    bin_ = nc.declare_dram_parameter("bin", [128, BW], BF16, isOutput=False)
    fin = nc.declare_dram_parameter("fin", [1, FW], F32, isOutput=False)
    out = nc.declare_dram_parameter("out", [1, 1], F32, isOutput=True)

    with tile.TileContext(nc) as tc:
        with (
            tc.tile_pool(name="wts", bufs=1) as wp,
            tc.tile_pool(name="work", bufs=2) as kp,
            tc.tile_pool(name="ps1", bufs=1, space="PSUM") as pp1,
            tc.tile_pool(name="ps2", bufs=3, space="PSUM") as pp2,
        ):
            # consts+centers land first (unblock prologue); x in 3 chunks.
            # Each dma_start's descriptor generation serializes on its
            # issuing sequencer (~2-3us for a [128,*] transfer), so spread
            # the loads across idle engines' DGE paths to issue in parallel.
            bsb = wp.tile([128, XT_O], BF16, tag="bsb")
            nc.sync.dma_start(bsb[:], bin_[:, 0:XT_O])
            fsb = wp.tile([1, FW], F32, tag="fsb")
            nc.sync.dma_start(fsb[:], fin[:])
            # x arrives per ANCHOR chunk (m-major): chunk m's GEMM only
            # waits for its own 98KB, not the whole shard
            xtiles = []
            for m in range(MCH):
                xm = wp.tile([128, KD * 128], BF16, tag=f"xm{m}")
                nc.sync.dma_start(
                    xm[:], bin_[:, XT_O + m * KD * 128 :
                                 XT_O + (m + 1) * KD * 128]
                )
                xtiles.append(xm)

            cwt_b = bsb[:, CWT_O : CWT_O + KD * C]
            ct_b = bsb[:, CT_O : CT_O + KD * C]
            t_b = bsb[:, T_O : T_O + MCH]
            penrow_f = fsb[0:1, 0:C]

            # ---- constants built on device (gpsimd is idle early) ----
            ones_b = wp.tile([128, 128], BF16, tag="ones_b")
            nc.gpsimd.memset(ones_b[:], 1.0)
            ones_f = wp.tile([128, 1], F32, tag="ones_f")
            nc.gpsimd.memset(ones_f[:], 1.0)
            iota_i = wp.tile([128, C], mybir.dt.int32, tag="iota_i")
            nc.gpsimd.iota(iota_i[:], pattern=[[1, C]], base=0,
                           channel_multiplier=0)
            iota_b = wp.tile([128, C], BF16, tag="iota_b")
            nc.gpsimd.tensor_copy(iota_b[:], iota_i[:])
            eye_b = wp.tile([C, C], BF16, tag="eye_b")
            nc.gpsimd.affine_select(
                eye_b[:], ones_b[0:C, 0:C], pattern=[[1, C]],
                compare_op=ALU.is_equal, fill=0.0, base=0,
                channel_multiplier=-1,
            )
            penfull = wp.tile([C, C], BF16, tag="penfull")
            nc.gpsimd.memset(penfull[:], PEN)
            eyepen_b = wp.tile([C, C], BF16, tag="eyepen_b")
            nc.gpsimd.affine_select(
                eyepen_b[:], penfull[:], pattern=[[1, C]],
                compare_op=ALU.is_equal, fill=0.0, base=0,
                channel_multiplier=-1,
            )

            # PE warm-up: HAM clock-gates a cold PE to 1.2GHz and needs
            # ~3.4us of sustained activity to ungate. Burn dummy matmuls on
            # the all-ones tile during the otherwise-dead DMA wait so the
            # real GEMMs run at 2.4GHz.
            warm_ps = pp1.tile([128, C], F32, tag="loss")
            for _ in range(24):
                nc.tensor.matmul(warm_ps[:], ones_b[:], ones_b[:, 0:C])

            # ---- center-side prep (bf16) ----
            w2b = wp.tile([128, KD * C], BF16, tag="w2b")
            nc.scalar.activation(w2b[:], cwt_b, AF.Exp, scale=LN2)
            # dummy sqrt: pulls the sqrt ACT table load off the critical
            # path; reads w2b so it lands after the Exp (exp-table) use.
            sqdummy = wp.tile([1, 1], F32, tag="sqdummy")
            nc.scalar.sqrt(sqdummy[:], w2b[0:1, 0:1])

            csqb = wp.tile([128, KD * C], BF16, tag="csqb")
            nc.scalar.square(csqb[:], ct_b)
            cm2b = wp.tile([128, KD * C], BF16, tag="cm2b")
            nc.vector.tensor_scalar(cm2b[:], ct_b, -2.0, None, op0=ALU.mult)
            m2b = wp.tile([128, KD * C], BF16, tag="m2b")
            nc.vector.tensor_tensor(m2b[:], w2b[:], cm2b[:], op=ALU.mult)
            wsqb = wp.tile([128, KD * C], BF16, tag="wsqb")
            nc.vector.tensor_tensor(wsqb[:], w2b[:], csqb[:], op=ALU.mult)

            # one-hot masks (only need iota/t -> very early)
            ohw = wp.tile([128, MCH * C], F32, tag="ohw")
            oh3 = ohw[:].rearrange("p (m c) -> p m c", c=C)
            nc.vector.tensor_tensor(
                oh3, iota_b[:, None, :].broadcast_to([128, MCH, C]),
                t_b[:, :, None].broadcast_to([128, MCH, C]), op=ALU.is_equal
            )
            ohpw = wp.tile([128, MCH * C], F32, tag="ohpw")
            nc.vector.tensor_scalar(ohpw[:], ohw[:], PEN, None, op0=ALU.mult)

            # a[c] = sum_d w2*c^2, + absent-class penalty row -> bf16
            psum_arow = pp1.tile([1, C], F32, tag="arow")
            for k in range(KD):
                nc.tensor.matmul(
                    psum_arow[:], ones_b[:, 0:1],
                    wsqb[:, k * C : (k + 1) * C],
                    start=(k == 0), stop=(k == KD - 1),
                )
            arowb = wp.tile([1, C], BF16, tag="arowb")
            nc.vector.tensor_tensor(
                arowb[:], psum_arow[:], penrow_f, op=ALU.add
            )

            # ---- cd2 GEMM [100,100]: shared center operands + PEN*I ----
            psum_cd2 = pp1.tile([C, C], F32, tag="cd2")
            for k in range(KD):
                nc.tensor.matmul(
                    psum_cd2[:], m2b[:, k * C : (k + 1) * C],
                    ct_b[:, k * C : (k + 1) * C],
                    start=(k == 0), stop=False,
                )
                nc.tensor.matmul(
                    psum_cd2[:], w2b[:, k * C : (k + 1) * C],
                    csqb[:, k * C : (k + 1) * C],
                    start=False, stop=False,
                )
            nc.tensor.matmul(
                psum_cd2[:], eyepen_b[:], eye_b[:],
                start=False, stop=False,
            )
            nc.tensor.matmul(
                psum_cd2[:], arowb[:], ones_b[0:1, 0:C],
                start=False, stop=True,
            )
            # min over j, then clip at 0 (max is monotone, so this equals
            # the reference's clip-then-min) -- tiny [C,1] clip
            cdmin2 = wp.tile([C, 1], F32, tag="cdmin2")
            nc.vector.tensor_reduce(
                cdmin2[:], psum_cd2[:], axis=mybir.AxisListType.X, op=ALU.min
            )
            cdmin2c = wp.tile([C, 1], F32, tag="cdmin2c")
            nc.vector.tensor_scalar(cdmin2c[:], cdmin2[:], 0.0, None,
                                    op0=ALU.max)
            cdminb = wp.tile([C, 1], BF16, tag="cdminb")
            nc.scalar.sqrt(cdminb[:], cdmin2c[:])
            # transpose to a row, broadcast down 128 partitions
            psum_cdrow = pp1.tile([1, C], F32, tag="cdrow")
            nc.tensor.matmul(psum_cdrow[:], cdminb[:], eye_b[:])
            cdrowb = wp.tile([1, C], BF16, tag="cdrowb")
            nc.vector.tensor_copy(cdrowb[:], psum_cdrow[:])
            psum_bc = pp1.tile([128, C], F32, tag="bcast")
            nc.tensor.matmul(psum_bc[:], ones_b[0:1, :], cdrowb[:])
            cdbf = wp.tile([128, C], F32, tag="cdbf")
            nc.vector.tensor_copy(cdbf[:], psum_bc[:])

            # cc[i] = centers_dist[t_i] (dc2-independent -> overlapped)
            cctw = kp.tile([128, MCH * C], F32, tag="cctw")
            nc.vector.tensor_tensor(
                cctw[:].rearrange("p (m c) -> p m c", c=C),
                cdbf[:, None, :].broadcast_to([128, MCH, C]), oh3,
                op=ALU.mult,
            )
            cc4 = kp.tile([128, MCH], F32, tag="cc4")
            nc.vector.tensor_reduce(
                cc4[:], cctw[:].rearrange("p (m c) -> p m c", c=C),
                axis=mybir.AxisListType.X, op=ALU.add,
            )

            # x^2 per anchor chunk (pipelines with the x DMA chunks)
            xsqtiles = []
            for m in range(MCH):
                xsq = wp.tile([128, KD * 128], BF16, tag=f"xsq{m}")
                nc.scalar.square(xsq[:], xtiles[m][:])
                xsqtiles.append(xsq)

            # ---- big GEMM + per-chunk mining straight out of PSUM ----
            an2 = kp.tile([128, MCH], F32, tag="an2")
            ap2 = kp.tile([128, MCH], F32, tag="ap2")
            for m in range(MCH):
                psum_dc2 = pp2.tile([128, C], F32, tag="dc2")
                for k in range(KD):
                    nc.tensor.matmul(
                        psum_dc2[:],
                        xtiles[m][:, k * 128 : (k + 1) * 128],
                        m2b[:, k * C : (k + 1) * C],
                        start=(k == 0), stop=False,
                    )
                for k in range(KD):
                    nc.tensor.matmul(
                        psum_dc2[:],
                        xsqtiles[m][:, k * 128 : (k + 1) * 128],
                        w2b[:, k * C : (k + 1) * C],
                        start=False, stop=False,
                    )
                nc.tensor.matmul(
                    psum_dc2[:], ones_b[0:1, :], arowb[:],
                    start=False, stop=True,
                )
                antm = kp.tile([128, C], F32, tag="antm")
                nc.vector.tensor_tensor(
                    antm[:], psum_dc2[:], ohpw[:, m * C : (m + 1) * C],
                    op=ALU.add,
                )
                nc.vector.tensor_reduce(
                    an2[:, m : m + 1], antm[:],
                    axis=mybir.AxisListType.X, op=ALU.min,
                )
                aptm = kp.tile([128, C], F32, tag="aptm")
                nc.vector.tensor_tensor(
                    aptm[:], psum_dc2[:], ohw[:, m * C : (m + 1) * C],
                    op=ALU.mult,
                )
                nc.vector.tensor_reduce(
                    ap2[:, m : m + 1], aptm[:],
                    axis=mybir.AxisListType.X, op=ALU.add,
                )

            # ---- loss_i = sqrt(ap2) + relu(cc - sqrt(an2)) ----
            an = kp.tile([128, MCH], F32, tag="an")
            nc.scalar.sqrt(an[:], an2[:])
            mrgin = kp.tile([128, MCH], F32, tag="mrgin")
            nc.vector.tensor_tensor(mrgin[:], cc4[:], an[:], op=ALU.subtract)
            mrg = kp.tile([128, MCH], F32, tag="mrg")
            relusum = kp.tile([128, 1], F32, tag="relusum")
            nc.scalar.activation(mrg[:], mrgin[:], AF.Relu,
                                 accum_out=relusum[:])
            ap = kp.tile([128, MCH], F32, tag="ap")
            apsum = kp.tile([128, 1], F32, tag="apsum")
            nc.scalar.activation(ap[:], ap2[:], AF.Sqrt, accum_out=apsum[:])
            losscol = kp.tile([128, 1], F32, tag="losscol")
            nc.vector.tensor_tensor(
                losscol[:], relusum[:], apsum[:], op=ALU.add
            )

            psum_loss = pp1.tile([1, 1], F32, tag="loss")
            nc.tensor.matmul(psum_loss[:], ones_f[:], losscol[:])
            res_sb = wp.tile([1, 1], F32, tag="res")
            nc.vector.tensor_copy(res_sb[:], psum_loss[:])
            nc.sync.dma_start(out[:], res_sb[:])

    nc.compile()
    return nc


_NC_CACHE: list = []


def _get_nc() -> bass.Bass:
    if not _NC_CACHE:
        _NC_CACHE.append(build_nc())
    return _NC_CACHE[0]


def make_in_maps(inputs, centers, centers_weights, targets):
    x = np.asarray(inputs, dtype=np.float32)
    c = np.asarray(centers, dtype=np.float32)
    cw = np.asarray(centers_weights, dtype=np.float32)
    t = np.asarray(targets).astype(np.int64)

    bconst = np.zeros((128, BW), dtype=np.float32)
    cT = c.T.reshape(KD, 128, C)
    cwT = cw.T.reshape(KD, 128, C)
    for k in range(KD):
        bconst[:, CT_O + k * C : CT_O + (k + 1) * C] = cT[k]
        bconst[:, CWT_O + k * C : CWT_O + (k + 1) * C] = cwT[k]

    fshared = np.zeros((1, FW), dtype=np.float32)
    present = np.zeros(C, dtype=bool)
    present[np.unique(t)] = True
    fshared[0, 0:C] = np.where(present, 0.0, PEN)

    xT = np.ascontiguousarray(x.T)                      # [D, B]

    in_maps = []
    for i in range(NCORES):
        rows = slice(i * ROWS, (i + 1) * ROWS)
        bcst = bconst.copy()
        # [m, p, k*128+a]: anchor-chunk-major packing of x.T
        xr = xT[:, rows].reshape(KD, 128, MCH, 128).transpose(2, 1, 0, 3)
        for m in range(MCH):
            bcst[:, XT_O + m * KD * 128 : XT_O + (m + 1) * KD * 128] = (
                xr[m].reshape(128, KD * 128)
            )
        ts = t[rows].astype(np.float32).reshape(MCH, 128)
        bcst[:, T_O : T_O + MCH] = ts.T
        in_maps.append({
            "bin": bcst.astype(ml_dtypes.bfloat16),
            "fin": fshared,
        })
    return in_maps


def kernel(inputs, centers, centers_weights, targets, epoch_number=None,
           **_ignored):
    nc = _get_nc()
    in_maps = make_in_maps(inputs, centers, centers_weights, targets)
    res = run_bass_kernel_spmd(nc, in_maps, core_ids=list(range(NCORES)))
    total = sum(float(r["out"][0, 0]) for r in res.results)
    return np.float32(total / B)
